# revision 70
# baseline (speedup 1.0000x reference)
"""AttentionPairBias Trainium2 kernel (8-core SPMD, row-sharded), v2.

Sharding: core c owns query rows i in [128c, 128c+128) and the matching z
rows. k/v shards are computed from each core's own rows and AllGathered.

Key structure (vs v1):
- z ships in TWO host-prepared layouts: natural f16 [i, j, z] (feeds the
  variance path) and transposed fp8e4 [z, j, i] (feeds the PE projection as
  the stationary operand) -- no device transposes, no psum->sbuf zt copies.
- LN mean-centering is folded into the wz weights on the host:
      bias_h = rs * (z . w_hat_h),  w_hat_h = znw*wz_h - (c1_h/DZ)*1
  so no mu corrections run on device. mu itself (needed only for the
  variance) comes free as a 17th ones-column of the PE projection.
- Sum z^2 comes from a Scalar-engine in-place Square over the natural tile
  plus one DVE tensor_scalar(accum_out) per j (4x DVE mode).
- rs folds the 1/64 fp8 weight prescale and 1/DZ via the Sqrt scale/bias.
- Attention is lag-pipelined in two 512-j slabs so slab 0 overlaps the
  second half of the z loop; exp produces softmax denominators via
  accum_out. No softmax max-subtraction (logits are O(1)).
"""
import numpy as np

import concourse.bass as bass
import concourse.tile as tile_mod
from concourse import mybir
from concourse.tile import TileContext
from concourse.masks import make_identity
from concourse.vector_clock import ScopedClock

F32 = mybir.dt.float32
F16 = mybir.dt.float16
F8 = mybir.dt.float8e4

S = 1024          # sequence length
DS = 1024         # model dim
H = 16            # heads
HD = 64           # head dim
DZ = 128          # pair dim
NCORES = 8
SI = S // NCORES  # 128 query rows per core

W8_SCALE = 64.0   # host prescale on w_hat so fp8 stays in normal range


# ---------------------------------------------------------------------------
# Framework patch: this walrus build accepts only ONE semaphore wait per
# instruction, but TileContext's final drain aggregates every outstanding sem
# wait onto a single SP Drain. Split the waits across a chain of Drains.
# ---------------------------------------------------------------------------
def _patched_drain_and_barrier(self, tick_clock, wait_clock):
    nc = self.nc
    drain_inst = nc.sync.drain()
    wait_clock.add_sem_waits(
        drain_inst.ins, ScopedClock({None: tick_clock.global_clock})
    )
    si = drain_inst.ins.sync_info
    if si is not None and si.on_wait is not None and len(si.on_wait) > 1:
        extra = list(si.on_wait[1:])
        del si.on_wait[1:]
        for w in extra:
            d2 = nc.sync.drain()
            si2 = d2.ins.sync_info
            if si2 is None:
                d2.ins.sync_info = mybir.SyncInfo(on_wait=[w], on_update=[])
            else:
                si2.on_wait.append(w)
    nc.all_engine_barrier()
    assert self.sems is not None
    popped = nc._tile_sem_poison_stack.pop()
    assert popped is self._sem_poison
    nc.clear_and_free_semaphores(list(self.sems.allocated().values()))
    nc.all_engine_barrier()


def _install_patches():
    tile_mod.TileContext._drain_and_barrier = _patched_drain_and_barrier


_install_patches()


def _split_multiwait(nc):
    """This walrus build accepts at most one semaphore wait per instruction;
    Tile emits more when an op depends on producers on several engines. Hoist
    all-but-one wait onto same-engine NOPs inserted just before."""
    for fn in nc.m.functions:
        for bb in fn.blocks:
            out = []
            changed = False
            for inst in bb.instructions:
                si = inst.sync_info
                if si is not None and si.on_wait is not None and len(si.on_wait) > 1:
                    extra = list(si.on_wait[:-1])
                    del si.on_wait[:-1]
                    for w in extra:
                        out.append(mybir.InstNoOp(
                            name=nc.get_next_instruction_name(),
                            engine=inst.engine,
                            bass_nofuse=True,
                            sync_info=mybir.SyncInfo(on_wait=[w], on_update=[]),
                        ))
                    changed = True
                out.append(inst)
            if changed:
                bb.instructions[:] = out


def _bcast(ap, dims, extra_offset=0):
    return bass.AP(tensor=ap.tensor, offset=ap.offset + extra_offset, ap=dims)


def build_nc(split_waits=True, debug_taps=False):
    nc = bass.Bass("TRN2", target_bir_lowering=False, debug=False,
                   num_devices=NCORES)

    zn16 = nc.dram_tensor("zn16", [SI, S, DZ], F16, kind="ExternalInput").ap()
    zt8 = nc.dram_tensor("zt8", [DZ, S, SI], F8, kind="ExternalInput").ap()
    sTi16 = nc.dram_tensor("sTi16", [DS, SI], F16, kind="ExternalInput").ap()
    sT16 = nc.dram_tensor("sT16", [DS, S], F16, kind="ExternalInput").ap()
    wqT16 = nc.dram_tensor("wqT16", [DS, DS], F16, kind="ExternalInput").ap()
    wkT16 = nc.dram_tensor("wkT16", [DS, DS], F16, kind="ExternalInput").ap()
    wvT16 = nc.dram_tensor("wvT16", [DS, DS], F16, kind="ExternalInput").ap()
    wgT16 = nc.dram_tensor("wgT16", [DS, DS], F16, kind="ExternalInput").ap()
    woT16 = nc.dram_tensor("woT16", [DS, DS], F16, kind="ExternalInput").ap()
    w8 = nc.dram_tensor("w8", [DZ, H + 1], F8, kind="ExternalInput").ap()
    bq8 = nc.dram_tensor("bq8", [DS, 1], F32, kind="ExternalInput").ap()
    out_sh = nc.dram_tensor("out_sh", [SI, DS], F32, kind="ExternalOutput").ap()

    taps = {}
    if debug_taps:
        for name, shape, dt in [
            ("d_ssq", [128, S], F32), ("d_mus", [128, S], F16),
            ("d_rs", [128, S], F32), ("d_eb", [128, S, H], F16),
            ("d_bias", [128, S, H], F16), ("d_qT", [128, KT := 8, SI], F16),
            ("d_kT", [128, 8, S], F16), ("d_g16", [128, DS], F16),
            ("d_sums", [128, H, 2], F32), ("d_og", [128, DS], F16),
        ]:
            taps[name] = nc.dram_tensor(name, shape, dt,
                                        kind="ExternalOutput").ap()

    with TileContext(nc, pool_alloc_mode="queue") as tc:
        _emit(nc, tc, zn16, zt8, sTi16, sT16, wqT16, wkT16, wvT16, wgT16,
              woT16, w8, bq8, out_sh, taps)
    if split_waits:
        _split_multiwait(nc)
    return nc


def _emit(nc, tc, zn16, zt8, sTi16, sT16, wqT16, wkT16, wvT16, wgT16,
          woT16, w8, bq8, out_sh, taps=None):
    from contextlib import ExitStack
    AL = mybir.AluOpType
    AF = mybir.ActivationFunctionType

    KT = 8            # 1024/128 d tiles
    G = 32            # j's per z group
    NG = S // G       # 32 groups
    JB2 = 16          # j's per P-hat psum bank (16*17 = 272 fp32 <= 512)
    BLK = 128         # j's per rs-finalize block
    SLAB = 512        # j's per attention slab (2 slabs, lag-pipelined)

    ctx = ExitStack()
    with ctx:
        consts = ctx.enter_context(tc.tile_pool(name="consts", bufs=1))
        persist = ctx.enter_context(tc.tile_pool(name="persist", bufs=1))

        ident16 = consts.tile([128, 128], F16)
        make_identity(nc, ident16)
        w8_sb = consts.tile([DZ, H + 1], F8)
        nc.sync.dma_start(out=w8_sb, in_=w8)
        bq_sb = consts.tile([128, KT], F32)
        nc.sync.dma_start(out=bq_sb, in_=bq8.rearrange("(m p) o -> p (m o)", p=128))
        eps_sb = consts.tile([128, 1], F32)
        nc.vector.memset(eps_sb, 4096.0 * 1e-5)

        # persistent SBUF tensors
        qT_sb = persist.tile([128, KT, SI], F16)    # [d-part, d-tile, i]
        g16 = persist.tile([128, DS], F16)          # [i, d]
        kT_sb = persist.tile([128, KT, S], F16)     # [hd-pair-part, m, j]
        v_sb = persist.tile([128, KT, DS], F16)     # [j-part, j-tile, d]
        ssq = persist.tile([128, S], F32)           # sum z^2 per (i, j)
        rs = persist.tile([128, S], F32)            # rsqrt(var+eps)/64
        # P16e holds [P-hat | 64*mu] per j (17 cols); the P-hat part is later
        # overwritten with exp(bias). bias (rs * P-hat) only lives per-block
        # between gpsimd and the exp, so it rides a small ring.
        P16e = persist.tile([128, S, H + 1], F16)
        sums2 = persist.tile([128, H, 2], F32)      # attn row sums per slab
        inv = persist.tile([128, H], F32)
        og16 = persist.tile([128, DS], F16)
        ogT_sb = persist.tile([128, KT, SI], F16)

        zpool = ctx.enter_context(tc.tile_pool(name="zpool", bufs=2))
        ztpool = ctx.enter_context(tc.tile_pool(name="ztpool", bufs=2))
        scrpool = ctx.enter_context(tc.tile_pool(name="scrpool", bufs=2))
        wpool = ctx.enter_context(tc.tile_pool(name="wpool", bufs=3))
        attnp = ctx.enter_context(tc.tile_pool(name="attnp", bufs=2))
        ppsum = ctx.enter_context(
            tc.tile_pool(name="ppsum", bufs=2, space="PSUM"))
        scps = ctx.enter_context(
            tc.tile_pool(name="scps", bufs=2, space="PSUM"))
        atps = ctx.enter_context(
            tc.tile_pool(name="atps", bufs=2, space="PSUM"))
        ops = ctx.enter_context(
            tc.tile_pool(name="ops", bufs=1, space="PSUM"))

        obs = {}                                    # per-slab o accumulators
        o0_sb = persist.tile([128, DS], F32)        # slab-0 o evac / out stage

        # ---------------- Phase A: stage s + weights ----------------
        # k/v are computed LOCALLY for all 1024 rows (fp8) instead of an
        # AllGather -- the collective's ~100us latency stalled the pipeline.
        # Host folds: wq,bq carry 1/(8*32); wo carries 1/32; wk/wv are x32.
        sTi_sb = consts.tile([128, KT, SI], F16)
        nc.sync.dma_start(
            out=sTi_sb, in_=sTi16.rearrange("(m p) n -> p m n", p=128))
        sT_sb = consts.tile([128, KT, S], F16)
        nc.sync.dma_start(
            out=sT_sb, in_=sT16.rearrange("(m p) n -> p m n", p=128))
        wv_sb = consts.tile([128, KT, DS], F16)
        nc.sync.dma_start(
            out=wv_sb, in_=wvT16.rearrange("(m p) n -> p m n", p=128))
        wk_half = [None, None]

        def load_w_half(wT16, n, eng=None):
            wh = wpool.tile([128, KT, 512], F16, tag="w")
            (eng or nc.gpsimd).dma_start(
                out=wh,
                in_=wT16.rearrange("(m p) n -> p m n", p=128)[
                    :, :, 512 * n:512 * (n + 1)])
            return wh

        def k_unit(n, jt):
            # k rows for j-tile jt, output cols [512n, 512n+512)
            kp = scps.tile([128, 512], F32, tag="scp")
            for kk in range(KT):
                nc.tensor.matmul(kp, sT_sb[:, kk, 128 * jt:128 * (jt + 1)],
                                 wk_half[n][:, kk, :],
                                 start=(kk == 0), stop=(kk == KT - 1))
            kch = attnp.tile([128, 512], F16, tag="kch")
            nc.scalar.activation(kch, kp, AF.Copy)
            ktp = atps.tile([128, KT, 128], F16, tag="atb")
            for mm in range(4):
                nc.tensor.transpose(ktp[:, mm, :],
                                    kch[:, 128 * mm:128 * (mm + 1)], ident16)
            kt_out = _bcast(kT_sb, [list(kT_sb.ap[0]), [S, 4], [1, 128]],
                            extra_offset=(4 * n) * S + 128 * jt)
            nc.scalar.activation(kt_out, ktp[:, 0:4, :], AF.Copy)

        def v_unit(n, jt):
            vp = scps.tile([128, 512], F32, tag="scp")
            for kk in range(KT):
                nc.tensor.matmul(vp, sT_sb[:, kk, 128 * jt:128 * (jt + 1)],
                                 wv_sb[:, kk, 512 * n:512 * (n + 1)],
                                 start=(kk == 0), stop=(kk == KT - 1))
            nc.vector.tensor_copy(v_sb[:, jt, 512 * n:512 * (n + 1)], vp)

        def emit_qg():
            # qT[d, i] += bq  (wq, bq pre-scaled by 1/256 on host)
            for n in range(2):
                wh = load_w_half(wqT16, n)
                for mm in range(4):
                    m = 4 * n + mm
                    qp = scps.tile([128, 512], F32, tag="scp")
                    for k in range(KT):
                        nc.tensor.matmul(qp[:, 0:SI],
                                         wh[:, k, 128 * mm:128 * (mm + 1)],
                                         sTi_sb[:, k, :],
                                         start=(k == 0), stop=(k == KT - 1))
                    nc.vector.tensor_scalar(
                        out=qT_sb[:, m, :], in0=qp[:, 0:SI],
                        scalar1=bq_sb[:, m:m + 1], scalar2=None, op0=AL.add)
            # g = sigmoid(s_i @ wg^T)   [i, d]
            for n in range(2):
                wh = load_w_half(wgT16, n)
                gp = scps.tile([128, 512], F32, tag="scp")
                for k in range(KT):
                    nc.tensor.matmul(gp, sTi_sb[:, k, :], wh[:, k, :],
                                     start=(k == 0), stop=(k == KT - 1))
                nc.scalar.activation(g16[:, 512 * n:512 * (n + 1)], gp,
                                     AF.Sigmoid)

        wo_half = [None, None]  # loaded late through the weight ring

        # ---------------- helpers ----------------
        def finalize_block(b):
            # rs = rsqrt(var + eps) / 64 via: u = 32*ssq - mus^2;
            # sq = sqrt(u + 4096*eps); rs = 1/sq.   (mus = 64*mu, col H)
            jb = slice(BLK * b, BLK * (b + 1))
            muv = P16e[:, jb, H:H + 1]
            t = attnp.tile([128, BLK], F32, tag="fin")
            nc.vector.tensor_tensor(
                out=t.rearrange("p (a b) -> p a b", b=1), in0=muv, in1=muv,
                op=AL.mult)
            u = attnp.tile([128, BLK], F32, tag="fin")
            nc.vector.scalar_tensor_tensor(
                out=u, in0=ssq[:, jb], scalar=32.0, in1=t,
                op0=AL.mult, op1=AL.subtract)
            sq = attnp.tile([128, BLK], F32, tag="fin")
            nc.scalar.activation(sq, u, AF.Sqrt, bias=eps_sb)
            nc.vector.reciprocal(rs[:, jb], sq)
            # bias16 = rs * P16 on gpsimd (2048 els/row per block), then
            # P16 block is dead -> overwrite it with exp(bias) (Scalar) so
            # attention can use exp(qk)*exp(bias).  All h-major.
            rs_rep = _bcast(rs, [list(rs.ap[0]), [1, BLK], [0, H]],
                            extra_offset=BLK * b)
            bias_blk = scrpool.tile([128, BLK, H], F16, tag="bb")
            nc.gpsimd.tensor_tensor(
                out=bias_blk, in0=P16e[:, jb, 0:H], in1=rs_rep,
                op=AL.mult)
            nc.scalar.activation(P16e[:, jb, 0:H], bias_blk, AF.Exp)

        def attn_pair(s, h0):
            # two heads share one attnT psum bank and a single Scalar evac
            js = slice(SLAB * s, SLAB * (s + 1))
            if s not in obs:
                obs[s] = ops.tile([128, H, HD], F32, tag="ob",
                                  name=f"ob{s}")
            at2 = []
            for hh in range(2):
                h = h0 + hh
                m, p0 = h // 2, 64 * (h % 2)
                scp = scps.tile([128, 512], F32, tag="scp")
                nc.tensor.matmul(scp[:, 0:SLAB], qT_sb[p0:p0 + 64, m, :],
                                 kT_sb[p0:p0 + 64, m, js],
                                 start=True, stop=True)
                eq16 = attnp.tile([128, SLAB], F16, tag="eq")
                nc.scalar.activation(eq16, scp[:, 0:SLAB], AF.Exp)
                at16 = attnp.tile([128, SLAB], F16, tag=f"at{hh}",
                                  name=f"at16_{hh}")
                nc.vector.scalar_tensor_tensor(
                    out=at16, in0=eq16, scalar=1.0, in1=P16e[:, js, h],
                    op0=AL.bypass, op1=AL.mult,
                    accum_out=sums2[:, h, s:s + 1])
                at2.append(at16)
            atb = atps.tile([128, KT, 128], F16, tag="atb")
            nt = SLAB // 128
            for hh in range(2):
                for t in range(nt):
                    nc.tensor.transpose(atb[:, nt * hh + t, :],
                                        at2[hh][:, 128 * t:128 * (t + 1)],
                                        ident16)
            attnT = attnp.tile([128, 2 * nt, 128], F16, tag="atT")
            nc.scalar.activation(attnT, atb[:, 0:2 * nt, :], AF.Copy)
            # per-slab o accumulator: a start=True matmul clears the whole
            # bank's has_written bits, so cross-slab accumulation in PSUM
            # would lose earlier heads' partials. Combine slabs in SBUF.
            for hh in range(2):
                h = h0 + hh
                for t in range(nt):
                    nc.tensor.matmul(obs[s][:, h, :],
                                     attnT[:, nt * hh + t, :],
                                     v_sb[:, nt * s + t,
                                          HD * h:HD * (h + 1)],
                                     start=(t == 0), stop=(t == nt - 1))

        def fold_slab(s):
            # evacuate/accumulate this slab's o into SBUF and free the bank
            if s == 0:
                nc.scalar.activation(
                    o0_sb, obs[0].rearrange("p h d -> p (h d)"), AF.Copy)
            else:
                nc.vector.tensor_tensor(
                    out=o0_sb, in0=o0_sb,
                    in1=obs[s].rearrange("p h d -> p (h d)"), op=AL.add)

        # ---------------- Phase B: z loop (+ lag-fused slab 0) --------------
        for g in range(NG):
            j0 = G * g
            z16 = zpool.tile([128, G, DZ], F16, tag="z16")
            nc.sync.dma_start(out=z16, in_=zn16[:, j0:j0 + G, :])
            zt8g = ztpool.tile([128, G, SI], F8, tag="zt8")
            nc.sync.dma_start(out=zt8g, in_=zt8[:, j0:j0 + G, :])

            # P-hat (+ 64*mu in col 16) per j, 16 j's per psum bank
            # (bank tile is a full 512-fp32 bank; j's live at 17-col stride)
            for t in range(G // JB2):
                pbank = ppsum.tile([128, 512], F32, tag="pb")
                for jj in range(JB2):
                    nc.tensor.matmul(pbank[:, 17 * jj:17 * jj + H + 1],
                                     zt8g[:, JB2 * t + jj, :], w8_sb,
                                     start=True, stop=True)
                jsl = slice(j0 + JB2 * t, j0 + JB2 * (t + 1))
                pb_p = _bcast(pbank,
                              [list(pbank.ap[0]), [17, JB2], [1, H + 1]])
                nc.scalar.activation(P16e[:, jsl, :], pb_p, AF.Copy)

            # sum z^2 per j: square (1/3 Scalar, 2/3 DVE), then an in-place
            # pairwise-halving tree 128->8 (DVE mostly, gpsimd for some
            # groups) plus a final segmented reduce.
            # square z in place (A1 on Scalar either way; frees a pool)
            sq16 = z16
            if g % 3 == 0:
                nc.vector.tensor_tensor(out=sq16, in0=z16, in1=z16,
                                        op=AL.mult)
            else:
                nc.scalar.activation(sq16, z16, AF.Square)
            scr = scrpool.tile([128, G, 64], F16, tag="scr")
            nc.vector.tensor_tensor(          # 128 -> 64
                out=scr, in0=sq16[:, :, 0:64],
                in1=sq16[:, :, 64:128], op=AL.add)
            nc.vector.tensor_tensor(          # 64 -> 32 (in place)
                out=scr[:, :, 0:32], in0=scr[:, :, 0:32],
                in1=scr[:, :, 32:64], op=AL.add)
            nc.vector.tensor_tensor(          # 32 -> 16
                out=scr[:, :, 0:16], in0=scr[:, :, 0:16],
                in1=scr[:, :, 16:32], op=AL.add)
            nc.vector.tensor_tensor(          # 16 -> 8
                out=scr[:, :, 0:8], in0=scr[:, :, 0:8],
                in1=scr[:, :, 8:16], op=AL.add)
            nc.vector.tensor_reduce(          # 8 -> 1 per j
                out=ssq[:, j0:j0 + G].rearrange("p (a b) -> p a b", b=1),
                in_=scr[:, :, 0:8], axis=mybir.AxisListType.X, op=AL.add)

            if (g + 1) % (BLK // G) == 0:
                finalize_block((g + 1) // (BLK // G) - 1)
            if g == 0:
                wk_half[0] = load_w_half(wkT16, 0, eng=nc.sync)
            if g == 5:
                wk_half[1] = load_w_half(wkT16, 1, eng=nc.sync)
            if g < 16:
                k_unit(g // 8, g % 8)
                v_unit(g // 8, g % 8)
            if g == 14:
                emit_qg()
            if g == 24 or g == 26:
                wo_half[(g - 24) // 2] = load_w_half(woT16, (g - 24) // 2)
            if g >= 16 and g % 2 == 0:
                attn_pair(0, g - 16)

        # ---------------- Phase C: slab 1 tail + output ----------------
        fold_slab(0)
        for h0 in range(0, H, 2):
            attn_pair(1, h0)
        fold_slab(1)

        nc.vector.tensor_tensor(out=inv, in0=sums2[:, :, 0],
                                in1=sums2[:, :, 1], op=AL.add)
        nc.vector.reciprocal(inv, inv)
        for h in range(H):
            nc.vector.scalar_tensor_tensor(
                out=og16[:, HD * h:HD * (h + 1)],
                in0=o0_sb[:, HD * h:HD * (h + 1)],
                scalar=inv[:, h:h + 1], in1=g16[:, HD * h:HD * (h + 1)],
                op0=AL.mult, op1=AL.mult)

        ogb = atps.tile([128, KT, 128], F16, tag="atb")
        for t in range(KT):
            nc.tensor.transpose(ogb[:, t, :],
                                og16[:, 128 * t:128 * (t + 1)], ident16)
        nc.scalar.activation(ogT_sb.rearrange("p k n -> p (k n)"),
                             ogb.rearrange("p k n -> p (k n)"), AF.Copy)
        for n in range(2):
            op_ = scps.tile([128, 512], F32, tag="scp")
            for k in range(KT):
                nc.tensor.matmul(op_, ogT_sb[:, k, :],
                                 wo_half[n][:, k, :],
                                 start=(k == 0), stop=(k == KT - 1))
            nc.vector.tensor_copy(o0_sb[:, 512 * n:512 * (n + 1)], op_)
        nc.sync.dma_start(out=out_sh, in_=o0_sb)

        if taps:
            for name, tile in [
                ("d_ssq", ssq), ("d_rs", rs), ("d_eb", P16e[:, :, 0:H]),
                ("d_qT", qT_sb), ("d_kT", kT_sb),
                ("d_g16", g16), ("d_sums", sums2), ("d_og", og16),
            ]:
                nc.scalar.dma_start(out=taps[name], in_=tile)
            nc.scalar.dma_start(
                out=taps["d_mus"].rearrange("p (a b) -> p a b", b=1),
                in_=P16e[:, :, H:H + 1])


def prep_inputs(s, z, wq, bq, wk, wv, wg, z_norm_w, z_norm_b, wz, wo):
    """Host-side prep: shard + transpose/cast. Returns in_maps."""
    import ml_dtypes
    F8NP = mybir.dt.np(F8)

    s2 = np.asarray(s)[0]                     # [S, DS]
    sT = np.ascontiguousarray(s2.T).astype(np.float16)
    wqT = np.ascontiguousarray((np.asarray(wq) / 8.0).T).astype(np.float16)
    wkT = np.ascontiguousarray(np.asarray(wk).T).astype(np.float16)
    wvT = np.ascontiguousarray(np.asarray(wv).T).astype(np.float16)
    wgT = np.ascontiguousarray(np.asarray(wg).T).astype(np.float16)
    woT = np.ascontiguousarray(np.asarray(wo).T).astype(np.float16)
    bq8 = (np.asarray(bq) / 8.0).astype(np.float32)[:, None]

    # w_hat: fold z_norm_w and the mean-centering into wz; prescale by 64
    # so fp8e4 stays in normal range (rs on device carries the 1/64).
    w_tld = np.asarray(z_norm_w)[:, None] * np.asarray(wz).T  # [DZ, H]
    w_hat = w_tld - w_tld.mean(axis=0, keepdims=True)
    w8 = np.empty((DZ, H + 1), dtype=F8NP)
    w8[:, :H] = (w_hat * W8_SCALE).astype(F8NP)
    w8[:, H] = np.float32(0.5)  # ones column scaled: col = 64/DZ = 0.5
    # z_norm_b contributes a per-head constant -> drops under softmax.

    z0 = np.asarray(z)[0]                     # [S, S, DZ]

    in_maps = []
    for c in range(NCORES):
        i0 = SI * c
        zc8 = z0[i0:i0 + SI].astype(F8NP)     # [SI, S, DZ] quantized once
        zn16 = zc8.astype(np.float16)         # stats see the same values
        zt = np.ascontiguousarray(zc8.transpose(2, 1, 0))  # [DZ, S, SI]
        in_maps.append({
            "zn16": zn16, "zt8": zt,
            "sTi16": np.ascontiguousarray(sT[:, i0:i0 + SI]), "sT16": sT,
            "wqT16": wqT, "wkT16": wkT, "wvT16": wvT, "wgT16": wgT,
            "woT16": woT, "w8": w8, "bq8": bq8,
        })
    return in_maps


_NC_CACHE = None


def _get_nc():
    global _NC_CACHE
    if _NC_CACHE is None:
        _NC_CACHE = build_nc()
    return _NC_CACHE


def kernel(**inputs):
    from concourse.bass_utils import run_bass_kernel_spmd
    nc = _get_nc()
    in_maps = prep_inputs(**inputs)
    res = run_bass_kernel_spmd(nc, in_maps, core_ids=list(range(NCORES)))
    out = np.empty((1, S, DS), dtype=np.float32)
    for c in range(NCORES):
        out[0, SI * c:SI * (c + 1), :] = res.results[c]["out_sh"]
    return out
